# revision 1
# baseline (speedup 1.0000x reference)
"""SPINN left-chain TreeLSTM kernel for Trainium2 (8 NeuronCores).

Problem: B=256 batch of identical left-branching shift-reduce parses over
N=64 tokens: 63 sequential TreeLSTM reduces, each
    lstm_in = lh @ Wl + bl + rh @ Wr          (lh = accumulated h, rh = token h)
    c = tanh(a)*sig(i) + sig(f1)*lc + sig(f2)*rc ;  h = sig(o)*tanh(c)
Output: final h [256, 512].

Strategy: pure data-parallel over batch (32 rows/core, no collectives —
the ~5-10us/call collective floor would dwarf the ~3us step).
Per core, each step's matmuls use PE column-tiling: the four 128-unit
blocks of the 2560 gate columns are packed into the four 32-partition
column groups of the PE array, so PSUM holds gates as
[128 partitions = (unit_block q, batch b), 640 free = (gate, unit v)].
That gives full-width (128-partition) elementwise ops, and the [batch x
unit] "h" tile is returned to stationary ("hT") form with a single PE
transpose per step. The token-side (Wr) matmuls and the bias ride in the
same PSUM accumulation group but execute during the previous step's
elementwise window, off the critical path. All matmul inputs are bf16
(fp32 PSUM accumulation); c/gates stay fp32.

All data re-layout (weight gate/block interleave, token transposes) is
done host-side in numpy; the device program is a fully unrolled 63-step
straight-line Tile kernel.
"""
import sys

sys.path.insert(0, "/opt/trn_rl_repo")

import numpy as np
import ml_dtypes

BF16 = ml_dtypes.bfloat16
F32 = np.float32

SIZE = 512
B = 256
NTOK = 64
T = 127
NCORES = 8
BPC = B // NCORES          # 32 batch rows per core
NSTEP = NTOK - 1           # 63 reduces
# gate order in the arranged weight columns: sig-block [f1, f2, i, o], then a
GATE_PERM = [2, 3, 1, 4, 0]  # orig gate index (a,i,f1,f2,o)=(0..4) -> [f1,f2,i,o,a]

_CACHE = {}
# build-time ablation/experiment switches (timing experiments only; the
# shipped kernel uses the defaults)
CFG = {}


def _expected_transitions():
    tr = np.ones(T, dtype=np.int32)
    tr[0] = 0
    tr[1::2] = 0
    return np.tile(tr[None, :], (B, 1))


def _numpy_fallback(buffers, transitions, Wl, Wr, bl):
    """Exact numpy replication of the reference scan (safety net)."""
    buffers = np.asarray(buffers, F32)
    transitions = np.asarray(transitions)
    Wl = np.asarray(Wl, F32)
    Wr = np.asarray(Wr, F32)
    bl = np.asarray(bl, F32)
    Bn, Nn, D2 = buffers.shape
    size = D2 // 2
    Tn = transitions.shape[1]
    max_depth = (Tn + 1) // 2 + 1
    bidx = np.arange(Bn)
    stack = np.zeros((Bn, max_depth, D2), F32)
    ptr = np.zeros(Bn, np.int32)
    bptr = np.zeros(Bn, np.int32)

    def sig(x):
        return 1.0 / (1.0 + np.exp(-x))

    for t in range(Tn):
        tr = transitions[:, t]
        is_shift = tr == 0
        is_reduce = tr == 1
        right = stack[bidx, np.maximum(ptr - 1, 0)]
        left = stack[bidx, np.maximum(ptr - 2, 0)]
        lh, lc = left[:, :size], left[:, size:]
        rh, rc = right[:, :size], right[:, size:]
        lstm_in = lh @ Wl + bl + rh @ Wr
        a, i, f1, f2, o = np.split(lstm_in, 5, axis=1)
        c = np.tanh(a) * sig(i) + sig(f1) * lc + sig(f2) * rc
        h = sig(o) * np.tanh(c)
        reduced = np.concatenate([h, c], axis=1)
        shifted = buffers[bidx, np.minimum(bptr, Nn - 1)]
        new_item = np.where(is_shift[:, None], shifted, reduced)
        write_pos = np.where(is_shift, ptr, np.maximum(ptr - 2, 0))
        do_write = is_shift | is_reduce
        old = stack[bidx, write_pos]
        stack[bidx, write_pos] = np.where(do_write[:, None], new_item, old)
        ptr = ptr + np.where(is_shift, 1, np.where(is_reduce, -1, 0)).astype(np.int32)
        bptr = bptr + is_shift.astype(np.int32)
    top = stack[bidx, np.maximum(ptr - 1, 0)]
    return top[:, :size]


def _build_program(repeat=1):
    import concourse.bacc as bacc
    import concourse.tile as tile
    from concourse import mybir
    from contextlib import ExitStack

    nc = bacc.Bacc("TRN2", target_bir_lowering=False, debug=False)
    dt = mybir.dt
    AF = mybir.ActivationFunctionType

    RA = nc.declare_dram_parameter("RA", [NSTEP, 128, 128], dt.bfloat16, isOutput=False)
    RC = nc.declare_dram_parameter("RC", [NSTEP, 128, 128], dt.float32, isOutput=False)
    CB = nc.declare_dram_parameter("CB", [128, 256], dt.bfloat16, isOutput=False)
    C0 = nc.declare_dram_parameter("C0", [128, 128], dt.float32, isOutput=False)
    WA = nc.declare_dram_parameter("WA", [4, 128, 5120], dt.bfloat16, isOutput=False)
    BL = nc.declare_dram_parameter("BL", [1, 2592], dt.bfloat16, isOutput=False)
    OUT = nc.declare_dram_parameter("out", [128, 128], dt.float32, isOutput=True)

    PF = 3  # DMA prefetch depth in steps

    with tile.TileContext(nc) as tc, ExitStack() as ctx:
        wpool = ctx.enter_context(tc.tile_pool(name="wpool", bufs=1))
        consts = ctx.enter_context(tc.tile_pool(name="consts", bufs=1))
        rtp = ctx.enter_context(tc.tile_pool(name="rtp", bufs=PF + 1))
        lp = ctx.enter_context(tc.tile_pool(name="lp", bufs=PF + 1))
        ep = ctx.enter_context(tc.tile_pool(name="ep", bufs=2))
        htp = ctx.enter_context(tc.tile_pool(name="htp", bufs=2))
        psum = ctx.enter_context(tc.tile_pool(name="psum", bufs=int(CFG.get("psum_bufs", 2)), space="PSUM"))
        pst = ctx.enter_context(tc.tile_pool(name="pst", bufs=2, space="PSUM"))
        kwp = ctx.enter_context(tc.tile_pool(name="kwp", bufs=1, space="PSUM"))

        # ---- constants / weights
        W_t = wpool.tile([128, 4 * 5120], dt.bfloat16, name="W_t")
        for kc in range(4):
            half = 5120 // 2
            nc.sync.dma_start(W_t[:, 5120 * kc:5120 * kc + half], WA[kc, :, 0:half])
            nc.sync.dma_start(W_t[:, 5120 * kc + half:5120 * (kc + 1)], WA[kc, :, half:5120])
        cb_t = consts.tile([128, 256], dt.bfloat16, name="cb_t")
        nc.sync.dma_start(cb_t[:], CB[:])
        lh0_t = cb_t[:, 0:128]
        id_t = cb_t[:, 128:256]
        bl_t = consts.tile([1, 2592], dt.bfloat16, name="bl_t")
        nc.sync.dma_start(bl_t[:], BL[:])
        ones_t = bl_t[:, 0:32]
        onef_t = consts.tile([1, 2], dt.float32, name="onef_t")
        nc.vector.memset(onef_t[:], 1.0)
        if CFG.get("salt"):
            # harmless const write: differentiates the BIR hash so compiler-flag
            # experiments don't hit the NEFF cache
            salt_t = consts.tile([1, 2], dt.float32, name="salt_t")
            nc.vector.memset(salt_t[:], float(CFG["salt"]))

        def wl(kc, lo, hi):
            return W_t[:, 5120 * kc + lo:5120 * kc + hi]

        def wr(kc, lo, hi):
            return W_t[:, 5120 * kc + 2560 + lo:5120 * kc + 2560 + hi]

        def chain():
            # ---- per-step tiles, created at prefetch time
            RT = {}   # step -> rhT stationary tile bf16 [128,128]
            L = {}    # step -> [lc | rc] f32 [128, 256]
            _emit_chain(nc, tc, mybir, RT, L, rtp, lp, ep, htp, psum, pst,
                        RA, RC, C0, OUT, wl, wr, bl_t, ones_t, lh0_t, id_t, PF,
                        kwp, onef_t)

        if repeat == 1:
            chain()
        else:
            with tc.For_i(0, repeat, 1):
                chain()

    nc.finalize()
    return nc


def _emit_chain(nc, tc, mybir, RT, L, rtp, lp, ep, htp, psum, pst,
                RA, RC, C0, OUT, wl, wr, bl_t, ones_t, lh0_t, id_t, PF,
                kwp, onef_t):
    dt = mybir.dt
    AF = mybir.ActivationFunctionType
    if True:
        kw_t = None if CFG.get("no_kw") else kwp.tile([1, 8], dt.float32, name="kw_t")

        def keep_warm(j, src_ap):
            """Tiny PE matmul gated on a mid-window elementwise result: keeps
            the HAM activity monitor from re-throttling the PE clock during
            the post-matmul dependency chain."""
            if kw_t is None:
                return
            nc.tensor.matmul(kw_t[0:1, j:j + 1], onef_t[0:1, 0:1], src_ap,
                             start=True, stop=True)
        def prefetch(k):
            if k > NSTEP:
                return
            rt = rtp.tile([128, 128], dt.bfloat16, name=f"rt{k}", tag="rt")
            nc.sync.dma_start(rt[:], RA[k - 1])
            RT[k] = rt
            lt = lp.tile([128, 256], dt.float32, name=f"l{k}", tag="l")
            nc.gpsimd.dma_start(lt[:, 128:256], RC[k - 1])
            L[k] = lt

        # prologue: prefetch steps 1..PF, plus initial lc
        for k in range(1, PF + 1):
            prefetch(k)
        nc.sync.dma_start(L[1][:, 0:128], C0[:])

        def r_group(k, P):
            """Token-side (Wr) + bias matmuls for step k into psum P.

            First writers of each psum region carry start=True. The Wl group
            finishes each region unless it is ablated (CFG['no_wl'])."""
            if CFG.get("no_wr"):
                return
            rt = RT[k]
            last = bool(CFG.get("no_wl"))
            if CFG.get("share"):
                for kc in range(4):
                    st = kc == 0
                    for q in range(4):
                        sp = last and kc == 3
                        tp = (0, 32 * q)
                        rs = rt[:, 32 * kc:32 * kc + 32]
                        nc.tensor.matmul(P[32 * q:32 * (q + 1), 0:512], rs,
                                         wr(kc, 640 * q, 640 * q + 512),
                                         start=st, stop=sp, tile_position=tp)
                        nc.tensor.matmul(P[32 * q:32 * (q + 1), 512:640], rs,
                                         wr(kc, 640 * q + 512, 640 * q + 640),
                                         start=st, stop=sp, tile_position=tp)
                if not CFG.get("no_bias"):
                    for q in range(4):
                        nc.tensor.matmul(P[32 * q:32 * (q + 1), 0:512], ones_t,
                                         bl_t[:, 32 + 640 * q:32 + 640 * q + 512],
                                         start=False, stop=False, tile_position=(0, 32 * q))
                        nc.tensor.matmul(P[32 * q:32 * (q + 1), 512:640], ones_t,
                                         bl_t[:, 32 + 640 * q + 512:32 + 640 * q + 640],
                                         start=False, stop=False, tile_position=(0, 32 * q))
                return
            if CFG.get("rr"):
                for kc in range(4):
                    st = kc == 0
                    for q in range(4):
                        nc.tensor.matmul(P[32 * q:32 * (q + 1), 0:512],
                                         rt[:, 32 * kc:32 * kc + 32],
                                         wr(kc, 640 * q, 640 * q + 512),
                                         start=st, stop=False, tile_position=(0, 32 * q))
                    for q in range(4):
                        nc.tensor.matmul(P[32 * q:32 * (q + 1), 512:640],
                                         rt[:, 32 * kc:32 * kc + 32],
                                         wr(kc, 640 * q + 512, 640 * q + 640),
                                         start=st, stop=False, tile_position=(0, 32 * q))
                if not CFG.get("no_bias"):
                    for q in range(4):
                        nc.tensor.matmul(P[32 * q:32 * (q + 1), 0:512], ones_t,
                                         bl_t[:, 32 + 640 * q:32 + 640 * q + 512],
                                         start=False, stop=False, tile_position=(0, 32 * q))
                    for q in range(4):
                        nc.tensor.matmul(P[32 * q:32 * (q + 1), 512:640], ones_t,
                                         bl_t[:, 32 + 640 * q + 512:32 + 640 * q + 640],
                                         start=False, stop=False, tile_position=(0, 32 * q))
                return
            for q in range(4):
                o_sig = P[32 * q:32 * (q + 1), 0:512]
                o_a = P[32 * q:32 * (q + 1), 512:640]
                tp = (0, 32 * q)
                for kc in range(4):
                    st = kc == 0
                    sp = last and kc == 3
                    nc.tensor.matmul(o_sig, rt[:, 32 * kc:32 * kc + 32],
                                     wr(kc, 640 * q, 640 * q + 512),
                                     start=st, stop=sp, tile_position=tp)
                    nc.tensor.matmul(o_a, rt[:, 32 * kc:32 * kc + 32],
                                     wr(kc, 640 * q + 512, 640 * q + 640),
                                     start=st, stop=sp, tile_position=tp)
                if not CFG.get("no_bias"):
                    nc.tensor.matmul(o_sig, ones_t,
                                     bl_t[:, 32 + 640 * q:32 + 640 * q + 512],
                                     start=False, stop=False, tile_position=tp)
                    nc.tensor.matmul(o_a, ones_t,
                                     bl_t[:, 32 + 640 * q + 512:32 + 640 * q + 640],
                                     start=False, stop=False, tile_position=tp)

        def wl_group(k, P, hT):
            """Accumulated-state (Wl) matmuls: a-block first so tanh(a) can
            start while sig-block matmuls still stream."""
            if CFG.get("no_wl"):
                return
            first = bool(CFG.get("no_wr"))
            if CFG.get("share"):
                for kc in range(4):
                    for q in range(4):
                        st = first and kc == 0
                        sp = kc == 3
                        tp = (0, 32 * q)
                        hs = hT[:, 32 * kc:32 * kc + 32]
                        nc.tensor.matmul(P[32 * q:32 * (q + 1), 512:640], hs,
                                         wl(kc, 640 * q + 512, 640 * q + 640),
                                         start=st, stop=sp, tile_position=tp)
                        nc.tensor.matmul(P[32 * q:32 * (q + 1), 0:256], hs,
                                         wl(kc, 640 * q, 640 * q + 256),
                                         start=st, stop=sp, tile_position=tp)
                        nc.tensor.matmul(P[32 * q:32 * (q + 1), 256:512], hs,
                                         wl(kc, 640 * q + 256, 640 * q + 512),
                                         start=st, stop=sp, tile_position=tp)
                return
            if CFG.get("rr"):
                blocks = [(512, 640), (0, 512)] if CFG.get("merged") else \
                    [(512, 640), (0, 256), (256, 512)]
                for lo, hi in blocks:
                    for kc in range(4):
                        for q in range(4):
                            nc.tensor.matmul(P[32 * q:32 * (q + 1), lo:hi],
                                             hT[:, 32 * kc:32 * kc + 32],
                                             wl(kc, 640 * q + lo, 640 * q + hi),
                                             start=(first and kc == 0), stop=(kc == 3),
                                             tile_position=(0, 32 * q))
                return
            for q in range(4):
                o_a = P[32 * q:32 * (q + 1), 512:640]
                tp = (0, 32 * q)
                for kc in range(4):
                    nc.tensor.matmul(o_a, hT[:, 32 * kc:32 * kc + 32],
                                     wl(kc, 640 * q + 512, 640 * q + 640),
                                     start=(first and kc == 0), stop=(kc == 3),
                                     tile_position=tp)
            for lo, hi in ((0, 256), (256, 512)):
                for q in range(4):
                    tp = (0, 32 * q)
                    o_sig = P[32 * q:32 * (q + 1), lo:hi]
                    for kc in range(4):
                        nc.tensor.matmul(o_sig, hT[:, 32 * kc:32 * kc + 32],
                                         wl(kc, 640 * q + lo, 640 * q + hi),
                                         start=(first and kc == 0), stop=(kc == 3),
                                         tile_position=tp)

        # ---- prologue for step 1: psum + R(1)
        P_cur = psum.tile([128, 640], dt.float32, name="p1", tag="P")
        r_group(1, P_cur)

        hT_prev = lh0_t  # step 1's "accumulated h" is token 0
        for k in range(1, NSTEP + 1):
            prefetch(k + PF)
            wl_group(k, P_cur, hT_prev)

            # next step's psum + token-side matmuls: emitted now so the PE
            # works on them during this step's elementwise window
            if k < NSTEP:
                P_nxt = psum.tile([128, 640], dt.float32, name=f"p{k + 1}", tag="P")
                r_group(k + 1, P_nxt)
            else:
                P_nxt = None

            # ---- elementwise
            ta = ep.tile([128, 128], dt.float32, name=f"ta{k}", tag="ta")
            nc.scalar.activation(ta[:], P_cur[:, 512:640], AF.Tanh)
            asig = ep.tile([128, 512], dt.float32, name=f"as{k}", tag="asig")
            nc.scalar.activation(asig[:, 0:256], P_cur[:, 0:256], AF.Sigmoid)
            nc.scalar.activation(asig[:, 256:512], P_cur[:, 256:512], AF.Sigmoid)

            m2 = ep.tile([128, 256], dt.float32, name=f"m2{k}", tag="m2")
            nc.vector.tensor_mul(m2[:], asig[:, 0:256], L[k][:, 0:256])
            m1 = ep.tile([128, 128], dt.float32, name=f"m1{k}", tag="m1")
            m1_eng = nc.vector if CFG.get("m1_dve") else nc.gpsimd
            m1_eng.tensor_mul(m1[:], ta[:], asig[:, 256:384])
            s1 = ep.tile([128, 128], dt.float32, name=f"s1{k}", tag="s1")
            nc.vector.tensor_add(s1[:], m2[:, 0:128], m2[:, 128:256])
            # c -> next step's lc slot
            if k < NSTEP:
                c_dst = L[k + 1][:, 0:128]
            else:
                c_fin = ep.tile([128, 128], dt.float32, name="c_fin", tag="cf")
                c_dst = c_fin[:]
            nc.vector.tensor_add(c_dst, s1[:], m1[:])

            keep_warm(0, m2[0:1, 0:1])
            keep_warm(1, s1[0:1, 0:1])

            tcL = ep.tile([128, 128], dt.float32, name=f"tc{k}", tag="tc")
            if CFG.get("no_tanh_c"):  # timing ablation only (wrong numerics)
                nc.vector.tensor_copy(tcL[:], c_dst)
            else:
                nc.scalar.activation(tcL[:], c_dst, AF.Tanh)
            keep_warm(2, tcL[0:1, 0:1])

            if k < NSTEP:
                hb = ep.tile([128, 128], dt.bfloat16, name=f"h{k}", tag="h")
                nc.vector.tensor_mul(hb[:], asig[:, 384:512], tcL[:])
                # transpose back to stationary form for the next matmul
                ht_ps = pst.tile([128, 128], dt.bfloat16, name=f"htp{k}", tag="htps")
                nc.tensor.transpose(ht_ps[:], hb[:], id_t)
                ht = htp.tile([128, 128], dt.bfloat16, name=f"ht{k}", tag="ht")
                nc.vector.tensor_copy(ht[:], ht_ps[:])
                hT_prev = ht
            else:
                hf = ep.tile([128, 128], dt.float32, name="hf", tag="hf")
                nc.vector.tensor_mul(hf[:], asig[:, 384:512], tcL[:])
                nc.sync.dma_start(OUT[:], hf[:])

            P_cur = P_nxt


def _pack_inputs(buffers, Wl, Wr, bl):
    """Host-side re-layout into the kernel's tensor formats."""
    buffers = np.asarray(buffers, F32)
    Wl = np.asarray(Wl, F32)
    Wr = np.asarray(Wr, F32)
    bl = np.asarray(bl, F32)

    # weights: W [512, 2560] -> [kc, p, q*640 + gi*128 + v], gates permuted
    def arr_w(W):
        w = W.reshape(4, 128, 5, 4, 128)[:, :, GATE_PERM, :, :]
        return np.ascontiguousarray(w.transpose(0, 1, 3, 2, 4).reshape(4, 128, 2560))

    WA = np.concatenate([arr_w(Wl), arr_w(Wr)], axis=2).astype(BF16)
    blp = bl.reshape(5, 4, 128)[GATE_PERM].transpose(1, 0, 2).reshape(2560)
    BLt = np.concatenate([np.ones(32, F32), blp])[None, :].astype(BF16)

    ident = np.eye(128, dtype=F32)
    in_maps = []
    for c in range(NCORES):
        bc = slice(BPC * c, BPC * (c + 1))
        bh = buffers[bc, 1:, :SIZE]          # [32, 63, 512]
        bcc = buffers[bc, 1:, SIZE:]         # [32, 63, 512]
        # RA[k, p, 32kc+b] = bh[b, k, kc*128+p]
        RAc = bh.reshape(BPC, NSTEP, 4, 128).transpose(1, 3, 2, 0).reshape(NSTEP, 128, 128)
        # RC[k, 32q+b, v] = bcc[b, k, q*128+v]
        RCc = bcc.reshape(BPC, NSTEP, 4, 128).transpose(1, 2, 0, 3).reshape(NSTEP, 128, 128)
        h0 = buffers[bc, 0, :SIZE]           # [32, 512]
        lh0T = h0.reshape(BPC, 4, 128).transpose(2, 1, 0).reshape(128, 128)
        c0 = buffers[bc, 0, SIZE:]
        C0c = c0.reshape(BPC, 4, 128).transpose(1, 0, 2).reshape(128, 128)
        in_maps.append({
            "RA": np.ascontiguousarray(RAc).astype(BF16),
            "RC": np.ascontiguousarray(RCc),
            "CB": np.concatenate([lh0T, ident], axis=1).astype(BF16),
            "C0": np.ascontiguousarray(C0c),
            "WA": WA,
            "BL": BLt,
        })
    return in_maps


def _run(in_maps, trace=False, **kw):
    from concourse.bass_utils import run_bass_kernel_spmd

    key = ("prog", tuple(sorted(CFG.items())))
    if key not in _CACHE:
        _CACHE[key] = _build_program()
    nc = _CACHE[key]
    return run_bass_kernel_spmd(nc, in_maps, list(range(NCORES)), trace=trace, **kw)


def kernel(buffers, transitions, Wl, Wr, bl):
    transitions = np.asarray(transitions)
    if transitions.shape != (B, T) or np.asarray(buffers).shape != (B, NTOK, 2 * SIZE) \
            or not np.array_equal(transitions, _expected_transitions()):
        # input doesn't match the compiled left-chain schedule: exact fallback
        return _numpy_fallback(buffers, transitions, Wl, Wr, bl)

    CFG.clear()
    CFG.update({"rr": True, "merged": True, "no_kw": True})
    if not np.any(np.asarray(bl)):
        CFG["no_bias"] = True
    in_maps = _pack_inputs(buffers, Wl, Wr, bl)
    res = _run(in_maps)
    out = np.empty((B, SIZE), F32)
    for c in range(NCORES):
        oc = res.results[c]["out"]  # [128, 128] = [(q,b), v]
        out[BPC * c:BPC * (c + 1)] = \
            oc.reshape(4, BPC, 128).transpose(1, 0, 2).reshape(BPC, SIZE)
    return out


if __name__ == "__main__":
    import reference as ref

    inputs = ref.setup_inputs()
    np_in = {k: np.asarray(v) for k, v in inputs.items()}
    got = kernel(**np_in)
    exp = np.asarray(ref.reference(**inputs))
    num = np.linalg.norm(got - exp)
    den = np.linalg.norm(exp)
    print("rms rel err:", num / den)
    print("absmax diff:", np.abs(got - exp).max())



# revision 16
# speedup vs baseline: 1.0034x; 1.0034x over previous
"""SPINN left-chain TreeLSTM kernel for Trainium2 (8 NeuronCores).

Problem: B=256 batch of identical left-branching shift-reduce parses over
N=64 tokens: 63 sequential TreeLSTM reduces, each
    lstm_in = lh @ Wl + bl + rh @ Wr          (lh = accumulated h, rh = token h)
    c = tanh(a)*sig(i) + sig(f1)*lc + sig(f2)*rc ;  h = sig(o)*tanh(c)
Output: final h [256, 512].

Strategy (v2): pure data-parallel over batch (32 rows/core, no collectives).
The token-side (Wr) contributions are chain-independent, so they are NOT
computed per step at M=32 (25% PE efficiency) as in v1; instead they are
precomputed at M=128 full-array efficiency, four steps x 32 batch rows per
matmul group, interleaved into the chain's PE idle windows, and evacuated
to SBUF as bf16 "GR" tiles. Each chain step then:
  1. stages GR[k] into PSUM with identity-selector col-tiled matmuls
     (start=True wave; the selector picks the step's 32 rows out of the
     4-step group and drops them at the right PSUM partition quarter),
  2. accumulates the Wl matmuls on top (start=False; 4 kc x 4 col-tiled
     quarters; PSUM [128 = (quarter q, batch b), 640 = gate*128 + v] with
     gate order [f1 f2 i a | o] split 512+128 across two banks),
  3. runs the LSTM elementwise with one fused sigmoid over [f1 f2 i],
     tanh(a) in-bank, sig(o) from the small bank,
  4. transposes tanh(c) and sig(o) separately on the PE (the sig(o)
     transpose is off the critical path) and multiplies them on DVE
     directly into the next step's bf16 stationary "hT" tile.
All matmul inputs bf16 (fp32 PSUM accumulation); c stays fp32.

All data re-layout is done host-side in numpy; the device program is a
fully unrolled 63-step straight-line Tile kernel.
"""
import sys

sys.path.insert(0, "/opt/trn_rl_repo")

import numpy as np
import ml_dtypes

BF16 = ml_dtypes.bfloat16
F32 = np.float32

SIZE = 512
B = 256
NTOK = 64
T = 127
NCORES = 8
BPC = B // NCORES          # 32 batch rows per core
NSTEP = NTOK - 1           # 63 reduces
NGRP = 16                  # 4-step precompute groups (last one padded)
# gate order in the arranged weight columns: [f1, f2, i, a | o]
# orig gate indices (a,i,f1,f2,o) = (0..4)
GATE_PERM = [2, 3, 1, 0, 4]

_CACHE = {}
CFG = {}


def _expected_transitions():
    tr = np.ones(T, dtype=np.int32)
    tr[0] = 0
    tr[1::2] = 0
    return np.tile(tr[None, :], (B, 1))


def _numpy_fallback(buffers, transitions, Wl, Wr, bl):
    """Exact numpy replication of the reference scan (safety net)."""
    buffers = np.asarray(buffers, F32)
    transitions = np.asarray(transitions)
    Wl = np.asarray(Wl, F32)
    Wr = np.asarray(Wr, F32)
    bl = np.asarray(bl, F32)
    Bn, Nn, D2 = buffers.shape
    size = D2 // 2
    Tn = transitions.shape[1]
    max_depth = (Tn + 1) // 2 + 1
    bidx = np.arange(Bn)
    stack = np.zeros((Bn, max_depth, D2), F32)
    ptr = np.zeros(Bn, np.int32)
    bptr = np.zeros(Bn, np.int32)

    def sig(x):
        return 1.0 / (1.0 + np.exp(-x))

    for t in range(Tn):
        tr = transitions[:, t]
        is_shift = tr == 0
        is_reduce = tr == 1
        right = stack[bidx, np.maximum(ptr - 1, 0)]
        left = stack[bidx, np.maximum(ptr - 2, 0)]
        lh, lc = left[:, :size], left[:, size:]
        rh, rc = right[:, :size], right[:, size:]
        lstm_in = lh @ Wl + bl + rh @ Wr
        a, i, f1, f2, o = np.split(lstm_in, 5, axis=1)
        c = np.tanh(a) * sig(i) + sig(f1) * lc + sig(f2) * rc
        h = sig(o) * np.tanh(c)
        reduced = np.concatenate([h, c], axis=1)
        shifted = buffers[bidx, np.minimum(bptr, Nn - 1)]
        new_item = np.where(is_shift[:, None], shifted, reduced)
        write_pos = np.where(is_shift, ptr, np.maximum(ptr - 2, 0))
        do_write = is_shift | is_reduce
        old = stack[bidx, write_pos]
        stack[bidx, write_pos] = np.where(do_write[:, None], new_item, old)
        ptr = ptr + np.where(is_shift, 1, np.where(is_reduce, -1, 0)).astype(np.int32)
        bptr = bptr + is_shift.astype(np.int32)
    top = stack[bidx, np.maximum(ptr - 1, 0)]
    return top[:, :size]


def _build_program(repeat=1):
    import concourse.bacc as bacc
    import concourse.tile as tile
    from concourse import mybir
    from contextlib import ExitStack

    nc = bacc.Bacc("TRN2", target_bir_lowering=False, debug=False)
    dt = mybir.dt
    AF = mybir.ActivationFunctionType

    with_bias = bool(CFG.get("bias"))

    RB = nc.declare_dram_parameter("RB", [NGRP, 4, 128, 128], dt.bfloat16, isOutput=False)
    RC = nc.declare_dram_parameter("RC", [NSTEP, 128, 128], dt.float32, isOutput=False)
    CB = nc.declare_dram_parameter("CB", [128, 256], dt.bfloat16, isOutput=False)
    C0 = nc.declare_dram_parameter("C0", [128, 128], dt.float32, isOutput=False)
    WA = nc.declare_dram_parameter("WA", [4, 128, 5120], dt.bfloat16, isOutput=False)
    BQ = nc.declare_dram_parameter("BQ", [128, 2560], dt.bfloat16, isOutput=False)
    OUT = nc.declare_dram_parameter("out", [128, 128], dt.float32, isOutput=True)
    DBG = None
    if CFG.get("dbg"):
        DBG = nc.declare_dram_parameter("dbg", [128, 640], dt.float32, isOutput=True)

    PF = 3        # rc DMA prefetch depth in steps
    GPF = 3       # RB group prefetch depth in groups
    PRE_AT = 2    # group g is precomputed during steps 4*(g-PRE_AT)+...

    pre_pro = bool(CFG.get("pre_pro"))
    nstep = int(CFG.get("nstep", NSTEP))

    with tile.TileContext(nc) as tc, ExitStack() as ctx:
        wpool = ctx.enter_context(tc.tile_pool(name="wpool", bufs=1))
        consts = ctx.enter_context(tc.tile_pool(name="consts", bufs=1))
        rbp = ctx.enter_context(tc.tile_pool(name="rbp", bufs=4 * NGRP if pre_pro else 4 * (GPF + 1)))
        grp = ctx.enter_context(tc.tile_pool(name="grp", bufs=4 * NGRP if pre_pro else 14))
        lp = ctx.enter_context(tc.tile_pool(name="lp", bufs=PF + 2))
        ep = ctx.enter_context(tc.tile_pool(name="ep", bufs=2))
        htp = ctx.enter_context(tc.tile_pool(name="htp", bufs=2))
        psum = ctx.enter_context(tc.tile_pool(name="psum", bufs=2, space="PSUM"))
        pre = ctx.enter_context(tc.tile_pool(name="pre", bufs=1, space="PSUM"))
        pst = ctx.enter_context(tc.tile_pool(name="pst", bufs=2, space="PSUM"))

        # ---- constants / weights
        W_t = wpool.tile([128, 4 * 5120], dt.bfloat16, name="W_t")
        for kc in range(4):
            half = 5120 // 2
            nc.sync.dma_start(W_t[:, 5120 * kc:5120 * kc + half], WA[kc, :, 0:half])
            nc.sync.dma_start(W_t[:, 5120 * kc + half:5120 * (kc + 1)], WA[kc, :, half:5120])
        cb_t = consts.tile([128, 256], dt.bfloat16, name="cb_t")
        nc.sync.dma_start(cb_t[:], CB[:])
        lh0_t = cb_t[:, 0:128]
        id_t = cb_t[:, 128:256]
        bq_t = consts.tile([128, 2560], dt.bfloat16, name="bq_t")
        nc.sync.dma_start(bq_t[:], BQ[:])

        def wl(kc, lo, hi):
            return W_t[:, 5120 * kc + lo:5120 * kc + hi]

        def wr(kc, lo, hi):
            return W_t[:, 5120 * kc + 2560 + lo:5120 * kc + 2560 + hi]

        def chain():
            RBT = {}   # (g, kc) -> stationary tile bf16 [128, 128]
            GRT = {}   # (g, q) -> gates_r bf16 [128, 640]
            L = {}     # step -> [lc | rc] f32 [128, 256]

            def fetch_rb(g):
                if g >= NGRP:
                    return
                for kc in range(4):
                    t = rbp.tile([128, 128], dt.bfloat16, name=f"rb{g}_{kc}", tag="rb")
                    nc.sync.dma_start(t[:], RB[g, kc])
                    RBT[(g, kc)] = t

            def fetch_rc(k):
                if k > NSTEP:
                    return
                lt = lp.tile([128, 256], dt.float32, name=f"l{k}", tag="l")
                nc.gpsimd.dma_start(lt[:, 128:256], RC[k - 1])
                L[k] = lt

            def pre_chunk(g, q, kcs):
                """Precompute gates_r for 4-step group g, quarter q, over the
                given kc sub-range. kcs=(0,1) opens the accumulation, (2,3)
                closes it and evacuates to a GR tile."""
                if g >= NGRP:
                    return
                if kcs[0] == 0:
                    pt = pre.tile([128, 640], dt.float32, name=f"pre{g}_{q}", tag="pre")
                    pre_chunk.cur = pt
                else:
                    pt = pre_chunk.cur
                for kc in kcs:
                    st = kc == 0
                    sp = kc == 3
                    nc.tensor.matmul(pt[:, 0:512], RBT[(g, kc)],
                                     wr(kc, 640 * q, 640 * q + 512),
                                     start=st, stop=sp, tile_position=(0, 0))
                    nc.tensor.matmul(pt[:, 512:640], RBT[(g, kc)],
                                     wr(kc, 640 * q + 512, 640 * q + 640),
                                     start=st, stop=sp, tile_position=(0, 0))
            def pre_evac(g, q):
                """Evacuate the finished precompute psum to a bf16 GR tile.
                Emitted separately (after the step's hT multiply) so the DVE
                FIFO doesn't stall the critical path on this 640-col copy."""
                if g >= NGRP:
                    return
                pt = pre_chunk.cur
                gr = grp.tile([128, 640], dt.bfloat16, name=f"gr{g}_{q}", tag="gr")
                if with_bias:
                    nc.vector.tensor_add(gr[:], pt[:], bq_t[:, 640 * q:640 * q + 640])
                else:
                    nc.vector.tensor_copy(gr[:], pt[:])
                GRT[(g, q)] = gr

            def staging(k):
                """Open step k's PSUM accumulation with gates_r[k]: identity-
                selector col-tiled matmuls. Returns the psum tile."""
                P = psum.tile([128, 640], dt.float32, name=f"p{k}", tag="P")
                g, s = (k - 1) // 4, (k - 1) % 4
                sel = id_t[:, 32 * s:32 * s + 32]
                for q in range(4):
                    # start=True on EVERY quarter: the has_written clear is
                    # scoped to the partition rows each MM writes, so each
                    # region's first writer must carry the flag itself.
                    gr = GRT[(g, q)]
                    nc.tensor.matmul(P[32 * q:32 * (q + 1), 0:512], sel, gr[:, 0:512],
                                     start=True, stop=False, tile_position=(0, 32 * q))
                    nc.tensor.matmul(P[32 * q:32 * (q + 1), 512:640], sel, gr[:, 512:640],
                                     start=True, stop=False, tile_position=(0, 32 * q))
                return P

            def wl_bank(k, P, hT, lo, hi):
                for kc in range(4):
                    for q in range(4):
                        nc.tensor.matmul(P[32 * q:32 * (q + 1), lo:hi],
                                         hT[:, 32 * kc:32 * kc + 32],
                                         wl(kc, 640 * q + lo, 640 * q + hi),
                                         start=False, stop=(kc == 3),
                                         tile_position=(0, 32 * q))

            # ---- prologue
            for g in range(NGRP if pre_pro else min(PRE_AT + GPF, NGRP)):
                fetch_rb(g)
            for k in range(1, PF + 1):
                fetch_rc(k)
            nc.sync.dma_start(L[1][:, 0:128], C0[:])
            for g in range(NGRP if pre_pro else PRE_AT):
                for q in range(4):
                    pre_chunk(g, q, (0, 1))
                    pre_chunk(g, q, (2, 3))
                    pre_evac(g, q)
            P_cur = staging(1)
            P_nxt = staging(2) if nstep >= 2 else None

            hT_prev = lh0_t
            for k in range(1, nstep + 1):
                fetch_rc(k + PF)
                if (k - 1) % 4 == 0:
                    fetch_rb(PRE_AT + GPF + (k - 1) // 4)

                # ---- Wl accumulation: big sig/tanh bank then o bank
                if CFG.get("dbg") != "no_wl":
                    wl_bank(k, P_cur, hT_prev, 0, 512)
                    wl_bank(k, P_cur, hT_prev, 512, 640)
                if DBG is not None and k == 1:
                    if CFG.get("dbg") == "gr":
                        dbg_sb = ep.tile([128, 640], dt.float32, name="dbg_sb", tag="dbg")
                        nc.vector.tensor_copy(dbg_sb[:], GRT[(0, 0)][:])
                        nc.sync.dma_start(DBG[:], dbg_sb[:])
                    else:
                        dbg_sb = ep.tile([128, 640], dt.float32, name="dbg_sb", tag="dbg")
                        nc.vector.tensor_copy(dbg_sb[:], P_cur[:])
                        nc.sync.dma_start(DBG[:], dbg_sb[:])

                # interleaved precompute: one (g, q) chunk per step, split in
                # half around the transposes so they don't stall the PE queue
                pg, pq = PRE_AT + (k - 1) // 4, (k - 1) % 4

                # ---- elementwise
                asig = ep.tile([128, 384], dt.float32, name=f"as{k}", tag="asig")
                nc.scalar.activation(asig[:, 0:384], P_cur[:, 0:384], AF.Sigmoid)
                ta = ep.tile([128, 128], dt.float32, name=f"ta{k}", tag="ta")
                nc.scalar.activation(ta[:], P_cur[:, 384:512], AF.Tanh)
                last = k == nstep
                so = ep.tile([128, 128],
                             dt.float32 if last else dt.bfloat16,
                             name=f"so{k}", tag="so_f" if last else "so")
                nc.scalar.activation(so[:], P_cur[:, 512:640], AF.Sigmoid)

                m2 = ep.tile([128, 256], dt.float32, name=f"m2{k}", tag="m2")
                nc.vector.tensor_mul(m2[:], asig[:, 0:256], L[k][:, 0:256])
                m1 = ep.tile([128, 128], dt.float32, name=f"m1{k}", tag="m1")
                nc.vector.tensor_mul(m1[:], ta[:], asig[:, 256:384])
                s1 = ep.tile([128, 128], dt.float32, name=f"s1{k}", tag="s1")
                nc.vector.tensor_add(s1[:], m2[:, 0:128], m2[:, 128:256])
                if not last:
                    c_dst = L[k + 1][:, 0:128]
                else:
                    c_fin = ep.tile([128, 128], dt.float32, name="c_fin", tag="cf")
                    c_dst = c_fin[:]
                nc.vector.tensor_add(c_dst, s1[:], m1[:])

                tc_t = ep.tile([128, 128],
                               dt.float32 if last else dt.bfloat16,
                               name=f"tc{k}", tag="tc_f" if last else "tc")
                nc.scalar.activation(tc_t[:], c_dst, AF.Tanh)

                if not last:
                    # PE: first precompute half, then transposes + staging,
                    # then the second half rides behind next step's wl
                    if not pre_pro:
                        pre_chunk(pg, pq, (0, 1))
                    pstt = pst.tile([128, 256], dt.bfloat16, name=f"pst{k}", tag="pst")
                    nc.tensor.transpose(pstt[:, 0:128], so[:], id_t)
                    if not pre_pro:
                        pre_chunk(pg, pq, (2, 3))
                    nc.tensor.transpose(pstt[:, 128:256], tc_t[:], id_t)
                    soT = ep.tile([128, 128], dt.bfloat16, name=f"soT{k}", tag="soT")
                    nc.vector.tensor_copy(soT[:], pstt[:, 0:128])
                    ht = htp.tile([128, 128], dt.bfloat16, name=f"ht{k}", tag="ht")
                    nc.vector.tensor_mul(ht[:], pstt[:, 128:256], soT[:])
                    hT_prev = ht
                    if not pre_pro:
                        pre_evac(pg, pq)
                    if k + 2 <= nstep:
                        P_new = staging(k + 2)
                    else:
                        P_new = None
                    P_cur, P_nxt = P_nxt, P_new
                else:
                    hf = ep.tile([128, 128], dt.float32, name="hf", tag="hf")
                    nc.vector.tensor_mul(hf[:], so[:], tc_t[:])
                    nc.sync.dma_start(OUT[:], hf[:])

        if repeat == 1:
            chain()
        else:
            with tc.For_i(0, repeat, 1):
                chain()

    nc.finalize()
    return nc


def _pack_inputs(buffers, Wl, Wr, bl):
    """Host-side re-layout into the kernel's tensor formats."""
    buffers = np.asarray(buffers, F32)
    Wl = np.asarray(Wl, F32)
    Wr = np.asarray(Wr, F32)
    bl = np.asarray(bl, F32)

    # weights: W [512, 2560] -> [kc, p, q*640 + gi*128 + v], gates permuted
    def arr_w(W):
        w = W.reshape(4, 128, 5, 4, 128)[:, :, GATE_PERM, :, :]
        return np.ascontiguousarray(w.transpose(0, 1, 3, 2, 4).reshape(4, 128, 2560))

    WA = np.concatenate([arr_w(Wl), arr_w(Wr)], axis=2).astype(BF16)
    blp = bl.reshape(5, 4, 128)[GATE_PERM].transpose(1, 0, 2).reshape(2560)
    BQ = np.tile(blp[None, :], (128, 1)).astype(BF16)

    ident = np.eye(128, dtype=F32)
    in_maps = []
    for c in range(NCORES):
        bc = slice(BPC * c, BPC * (c + 1))
        bh = buffers[bc, 1:, :SIZE]          # [32, 63, 512]
        bcc = buffers[bc, 1:, SIZE:]         # [32, 63, 512]
        # RB[g, kc, p, 32s+b] = bh[b, 4g+s, kc*128+p]
        bh2 = np.zeros((BPC, 4 * NGRP, 4, 128), F32)
        bh2[:, :NSTEP] = bh.reshape(BPC, NSTEP, 4, 128)
        RBc = bh2.reshape(BPC, NGRP, 4, 4, 128).transpose(1, 3, 4, 2, 0) \
                 .reshape(NGRP, 4, 128, 128)
        # RC[k, 32q+b, v] = bcc[b, k, q*128+v]
        RCc = bcc.reshape(BPC, NSTEP, 4, 128).transpose(1, 2, 0, 3).reshape(NSTEP, 128, 128)
        h0 = buffers[bc, 0, :SIZE]           # [32, 512]
        lh0T = h0.reshape(BPC, 4, 128).transpose(2, 1, 0).reshape(128, 128)
        c0 = buffers[bc, 0, SIZE:]
        C0c = c0.reshape(BPC, 4, 128).transpose(1, 0, 2).reshape(128, 128)
        in_maps.append({
            "RB": np.ascontiguousarray(RBc).astype(BF16),
            "RC": np.ascontiguousarray(RCc),
            "CB": np.concatenate([lh0T, ident], axis=1).astype(BF16),
            "C0": np.ascontiguousarray(C0c),
            "WA": WA,
            "BQ": BQ,
        })
    return in_maps


def _run(in_maps, trace=False, **kw):
    from concourse.bass_utils import run_bass_kernel_spmd

    key = ("prog", tuple(sorted(CFG.items())))
    if key not in _CACHE:
        _CACHE[key] = _build_program()
    nc = _CACHE[key]
    return run_bass_kernel_spmd(nc, in_maps, list(range(NCORES)), trace=trace, **kw)


def kernel(buffers, transitions, Wl, Wr, bl):
    transitions = np.asarray(transitions)
    if transitions.shape != (B, T) or np.asarray(buffers).shape != (B, NTOK, 2 * SIZE) \
            or not np.array_equal(transitions, _expected_transitions()):
        # input doesn't match the compiled left-chain schedule: exact fallback
        return _numpy_fallback(buffers, transitions, Wl, Wr, bl)

    CFG.clear()
    if np.any(np.asarray(bl)):
        CFG["bias"] = True
    in_maps = _pack_inputs(buffers, Wl, Wr, bl)
    res = _run(in_maps)
    out = np.empty((B, SIZE), F32)
    for c in range(NCORES):
        oc = res.results[c]["out"]  # [128, 128] = [(q,b), v]
        out[BPC * c:BPC * (c + 1)] = \
            oc.reshape(4, BPC, 128).transpose(1, 0, 2).reshape(BPC, SIZE)
    return out


if __name__ == "__main__":
    import reference as ref

    inputs = ref.setup_inputs()
    np_in = {k: np.asarray(v) for k, v in inputs.items()}
    got = kernel(**np_in)
    exp = np.asarray(ref.reference(**inputs))
    num = np.linalg.norm(got - exp)
    den = np.linalg.norm(exp)
    print("rms rel err:", num / den)
    print("absmax diff:", np.abs(got - exp).max())


# revision 25
# speedup vs baseline: 1.0415x; 1.0380x over previous
"""SPINN left-chain TreeLSTM kernel for Trainium2 (8 NeuronCores).

Problem: B=256 batch of identical left-branching shift-reduce parses over
N=64 tokens: 63 sequential TreeLSTM reduces, each
    lstm_in = lh @ Wl + bl + rh @ Wr          (lh = accumulated h, rh = token h)
    c = tanh(a)*sig(i) + sig(f1)*lc + sig(f2)*rc ;  h = sig(o)*tanh(c)
Output: final h [256, 512].

Strategy (v2): pure data-parallel over batch (32 rows/core, no collectives).
The token-side (Wr) contributions are chain-independent, so they are NOT
computed per step at M=32 (25% PE efficiency) as in v1; instead they are
precomputed at M=128 full-array efficiency, four steps x 32 batch rows per
matmul group, interleaved into the chain's PE idle windows, and evacuated
to SBUF as bf16 "GR" tiles. Each chain step then:
  1. stages GR[k] into PSUM with identity-selector col-tiled matmuls
     (start=True wave; the selector picks the step's 32 rows out of the
     4-step group and drops them at the right PSUM partition quarter),
  2. accumulates the Wl matmuls on top (start=False; 4 kc x 4 col-tiled
     quarters; PSUM [128 = (quarter q, batch b), 640 = gate*128 + v] with
     gate order [f1 f2 i a | o] split 512+128 across two banks),
  3. runs the LSTM elementwise with one fused sigmoid over [f1 f2 i],
     tanh(a) in-bank, sig(o) from the small bank,
  4. transposes tanh(c) and sig(o) separately on the PE (the sig(o)
     transpose is off the critical path) and multiplies them on DVE
     directly into the next step's bf16 stationary "hT" tile.
All matmul inputs bf16 (fp32 PSUM accumulation); c stays fp32.

All data re-layout is done host-side in numpy; the device program is a
fully unrolled 63-step straight-line Tile kernel.
"""
import sys

sys.path.insert(0, "/opt/trn_rl_repo")

import numpy as np
import ml_dtypes

BF16 = ml_dtypes.bfloat16
F32 = np.float32

SIZE = 512
B = 256
NTOK = 64
T = 127
NCORES = 8
BPC = B // NCORES          # 32 batch rows per core
NSTEP = NTOK - 1           # 63 reduces
NGRP = 16                  # 4-step precompute groups (last one padded)
# gate order in the arranged weight columns: [f1, f2, i, a | o]
# orig gate indices (a,i,f1,f2,o) = (0..4)
GATE_PERM = [2, 3, 1, 0, 4]

_CACHE = {}
CFG = {}


def _expected_transitions():
    tr = np.ones(T, dtype=np.int32)
    tr[0] = 0
    tr[1::2] = 0
    return np.tile(tr[None, :], (B, 1))


def _numpy_fallback(buffers, transitions, Wl, Wr, bl):
    """Exact numpy replication of the reference scan (safety net)."""
    buffers = np.asarray(buffers, F32)
    transitions = np.asarray(transitions)
    Wl = np.asarray(Wl, F32)
    Wr = np.asarray(Wr, F32)
    bl = np.asarray(bl, F32)
    Bn, Nn, D2 = buffers.shape
    size = D2 // 2
    Tn = transitions.shape[1]
    max_depth = (Tn + 1) // 2 + 1
    bidx = np.arange(Bn)
    stack = np.zeros((Bn, max_depth, D2), F32)
    ptr = np.zeros(Bn, np.int32)
    bptr = np.zeros(Bn, np.int32)

    def sig(x):
        return 1.0 / (1.0 + np.exp(-x))

    for t in range(Tn):
        tr = transitions[:, t]
        is_shift = tr == 0
        is_reduce = tr == 1
        right = stack[bidx, np.maximum(ptr - 1, 0)]
        left = stack[bidx, np.maximum(ptr - 2, 0)]
        lh, lc = left[:, :size], left[:, size:]
        rh, rc = right[:, :size], right[:, size:]
        lstm_in = lh @ Wl + bl + rh @ Wr
        a, i, f1, f2, o = np.split(lstm_in, 5, axis=1)
        c = np.tanh(a) * sig(i) + sig(f1) * lc + sig(f2) * rc
        h = sig(o) * np.tanh(c)
        reduced = np.concatenate([h, c], axis=1)
        shifted = buffers[bidx, np.minimum(bptr, Nn - 1)]
        new_item = np.where(is_shift[:, None], shifted, reduced)
        write_pos = np.where(is_shift, ptr, np.maximum(ptr - 2, 0))
        do_write = is_shift | is_reduce
        old = stack[bidx, write_pos]
        stack[bidx, write_pos] = np.where(do_write[:, None], new_item, old)
        ptr = ptr + np.where(is_shift, 1, np.where(is_reduce, -1, 0)).astype(np.int32)
        bptr = bptr + is_shift.astype(np.int32)
    top = stack[bidx, np.maximum(ptr - 1, 0)]
    return top[:, :size]


def _build_program(repeat=1):
    import concourse.bacc as bacc
    import concourse.tile as tile
    from concourse import mybir
    from contextlib import ExitStack

    nc = bacc.Bacc("TRN2", target_bir_lowering=False, debug=False)
    dt = mybir.dt
    AF = mybir.ActivationFunctionType

    with_bias = bool(CFG.get("bias"))

    RB = nc.declare_dram_parameter("RB", [NGRP, 4, 128, 128], dt.bfloat16, isOutput=False)
    RC = nc.declare_dram_parameter("RC", [NSTEP, 128, 128], dt.float32, isOutput=False)
    CB = nc.declare_dram_parameter("CB", [128, 256], dt.bfloat16, isOutput=False)
    C0 = nc.declare_dram_parameter("C0", [128, 128], dt.float32, isOutput=False)
    WA = nc.declare_dram_parameter("WA", [4, 128, 5120], dt.bfloat16, isOutput=False)
    BQ = nc.declare_dram_parameter("BQ", [128, 2560], dt.bfloat16, isOutput=False)
    OUT = nc.declare_dram_parameter("out", [128, 128], dt.float32, isOutput=True)
    DBG = None
    if CFG.get("dbg"):
        DBG = nc.declare_dram_parameter("dbg", [128, 640], dt.float32, isOutput=True)

    PF = 3        # rc DMA prefetch depth in steps
    GPF = 3       # RB group prefetch depth in groups
    PRE_AT = 2    # group g is precomputed during steps 4*(g-PRE_AT)+...

    pre_pro = bool(CFG.get("pre_pro"))
    nstep = int(CFG.get("nstep", NSTEP))

    with tile.TileContext(nc) as tc, ExitStack() as ctx:
        wpool = ctx.enter_context(tc.tile_pool(name="wpool", bufs=1))
        consts = ctx.enter_context(tc.tile_pool(name="consts", bufs=1))
        rbp = ctx.enter_context(tc.tile_pool(name="rbp", bufs=4 * NGRP if pre_pro else 4 * (GPF + 1)))
        grp = ctx.enter_context(tc.tile_pool(name="grp", bufs=4 * NGRP if pre_pro else 14))
        lp = ctx.enter_context(tc.tile_pool(name="lp", bufs=PF + 2))
        ep = ctx.enter_context(tc.tile_pool(name="ep", bufs=2))
        htp = ctx.enter_context(tc.tile_pool(name="htp", bufs=2))
        psum = ctx.enter_context(tc.tile_pool(name="psum", bufs=2, space="PSUM"))
        prea_p = ctx.enter_context(tc.tile_pool(name="prea", bufs=1, space="PSUM"))
        preb_p = ctx.enter_context(tc.tile_pool(name="preb", bufs=1, space="PSUM"))
        pst = ctx.enter_context(tc.tile_pool(name="pst", bufs=2, space="PSUM"))

        # ---- constants / weights
        W_t = wpool.tile([128, 4 * 5120], dt.bfloat16, name="W_t")
        for kc in range(4):
            half = 5120 // 2
            nc.sync.dma_start(W_t[:, 5120 * kc:5120 * kc + half], WA[kc, :, 0:half])
            nc.sync.dma_start(W_t[:, 5120 * kc + half:5120 * (kc + 1)], WA[kc, :, half:5120])
        cb_t = consts.tile([128, 256], dt.bfloat16, name="cb_t")
        nc.sync.dma_start(cb_t[:], CB[:])
        lh0_t = cb_t[:, 0:128]
        id_t = cb_t[:, 128:256]
        bq_t = consts.tile([128, 2560], dt.bfloat16, name="bq_t")
        nc.sync.dma_start(bq_t[:], BQ[:])

        def wl(kc, lo, hi):
            return W_t[:, 5120 * kc + lo:5120 * kc + hi]

        def wr(kc, lo, hi):
            return W_t[:, 5120 * kc + 2560 + lo:5120 * kc + 2560 + hi]

        def chain():
            RBT = {}   # (g, kc) -> stationary tile bf16 [128, 128]
            GRA = {}   # (g, q) -> gates_r sig/tanh cols bf16 [128, 512]
            GRO = {}   # (g, q) -> gates_r o cols bf16 [128, 128]
            L = {}     # step -> [lc | rc] f32 [128, 256]

            def fetch_rb(g):
                if g >= NGRP:
                    return
                for kc in range(4):
                    t = rbp.tile([128, 128], dt.bfloat16, name=f"rb{g}_{kc}", tag="rb")
                    nc.sync.dma_start(t[:], RB[g, kc])
                    RBT[(g, kc)] = t

            def fetch_rc(k):
                if k > NSTEP:
                    return
                lt = lp.tile([128, 256], dt.float32, name=f"l{k}", tag="l")
                nc.gpsimd.dma_start(lt[:, 128:256], RC[k - 1])
                L[k] = lt

            PA = {}    # (g, q) -> open psum tile for the 512-col A chunk
            PB = {}    # (g, q) -> open psum tile for the 128-col B (o) chunk

            def pre_a(g, q):
                """Precompute the sig/tanh 512 cols of gates_r for group g,
                quarter q (4 kc-accumulated M=128 matmuls into one bank)."""
                if g >= NGRP:
                    return
                pt = prea_p.tile([128, 512], dt.float32, name=f"pa{g}_{q}", tag="prea")
                for kc in range(4):
                    nc.tensor.matmul(pt[:], RBT[(g, kc)],
                                     wr(kc, 640 * q, 640 * q + 512),
                                     start=(kc == 0), stop=(kc == 3),
                                     tile_position=(0, 0))
                PA[(g, q)] = pt

            def pre_b(g, q):
                """Precompute the o-gate 128 cols of gates_r for (g, q)."""
                if g >= NGRP:
                    return
                pt = preb_p.tile([128, 128], dt.float32, name=f"pb{g}_{q}", tag="preb")
                for kc in range(4):
                    nc.tensor.matmul(pt[:], RBT[(g, kc)],
                                     wr(kc, 640 * q + 512, 640 * q + 640),
                                     start=(kc == 0), stop=(kc == 3),
                                     tile_position=(0, 0))
                PB[(g, q)] = pt

            def evac_a(g, q):
                """PSUM -> bf16 SBUF for the A chunk; emitted at the DVE tail
                end so the copy never blocks the m-chain or hT multiply."""
                if g >= NGRP:
                    return
                gr = grp.tile([128, 512], dt.bfloat16, name=f"gra{g}_{q}", tag="gra")
                if with_bias:
                    nc.vector.tensor_add(gr[:], PA[(g, q)][:], bq_t[:, 640 * q:640 * q + 512])
                else:
                    # ScalarE: keeps the 512-col copy off the busy DVE queue
                    nc.scalar.copy(gr[:], PA[(g, q)][:])
                GRA[(g, q)] = gr

            def evac_b(g, q):
                if g >= NGRP:
                    return
                gr = grp.tile([128, 128], dt.bfloat16, name=f"gro{g}_{q}", tag="gro")
                if with_bias:
                    nc.vector.tensor_add(gr[:], PB[(g, q)][:],
                                         bq_t[:, 640 * q + 512:640 * q + 640])
                else:
                    nc.scalar.copy(gr[:], PB[(g, q)][:])
                GRO[(g, q)] = gr

            def staging(k):
                """Open step k's PSUM accumulation with gates_r[k]: identity-
                selector col-tiled matmuls. Returns the psum tile."""
                P = psum.tile([128, 640], dt.float32, name=f"p{k}", tag="P")
                g, s = (k - 1) // 4, (k - 1) % 4
                sel = id_t[:, 32 * s:32 * s + 32]
                for q in range(4):
                    # start=True on EVERY quarter: the has_written clear is
                    # scoped to the partition rows each MM writes, so each
                    # region's first writer must carry the flag itself.
                    nc.tensor.matmul(P[32 * q:32 * (q + 1), 0:512], sel,
                                     GRA[(g, q)][:],
                                     start=True, stop=False, tile_position=(0, 32 * q))
                    nc.tensor.matmul(P[32 * q:32 * (q + 1), 512:640], sel,
                                     GRO[(g, q)][:],
                                     start=True, stop=False, tile_position=(0, 32 * q))
                return P

            def wl_bank(k, P, hT, lo, hi):
                for kc in range(4):
                    for q in range(4):
                        nc.tensor.matmul(P[32 * q:32 * (q + 1), lo:hi],
                                         hT[:, 32 * kc:32 * kc + 32],
                                         wl(kc, 640 * q + lo, 640 * q + hi),
                                         start=False, stop=(kc == 3),
                                         tile_position=(0, 32 * q))

            # ---- prologue
            for g in range(NGRP if pre_pro else min(PRE_AT + GPF, NGRP)):
                fetch_rb(g)
            for k in range(1, PF + 1):
                fetch_rc(k)
            nc.sync.dma_start(L[1][:, 0:128], C0[:])
            for g in range(NGRP if pre_pro else PRE_AT):
                for q in range(4):
                    pre_a(g, q)
                    evac_a(g, q)
                    pre_b(g, q)
                    evac_b(g, q)
            S = {1: staging(1)}

            hT_prev = lh0_t
            for k in range(1, nstep + 1):
                fetch_rc(k + PF)
                if (k - 1) % 4 == 0:
                    fetch_rb(PRE_AT + GPF + (k - 1) // 4)
                P_cur = S[k]

                # ---- Wl accumulation: big sig/tanh bank then o bank
                if CFG.get("dbg") != "no_wl":
                    wl_bank(k, P_cur, hT_prev, 0, 512)
                    wl_bank(k, P_cur, hT_prev, 512, 640)

                # deferred PE work from step k-1 (the o-cols precompute) and
                # step k+1's staging: scheduled here, right after this step's
                # wl group, so the PE FIFO never parks them between tanh(c)
                # and the next wl group
                if not pre_pro and k >= 2:
                    ppg, ppq = PRE_AT + (k - 2) // 4, (k - 2) % 4
                    pre_b(ppg, ppq)
                if k + 1 <= nstep:
                    S[k + 1] = staging(k + 1)

                if DBG is not None and k == 1:
                    if CFG.get("dbg") == "gr":
                        dbg_sb = ep.tile([128, 640], dt.float32, name="dbg_sb", tag="dbg")
                        nc.vector.tensor_copy(dbg_sb[:], GRT[(0, 0)][:])
                        nc.sync.dma_start(DBG[:], dbg_sb[:])
                    else:
                        dbg_sb = ep.tile([128, 640], dt.float32, name="dbg_sb", tag="dbg")
                        nc.vector.tensor_copy(dbg_sb[:], P_cur[:])
                        nc.sync.dma_start(DBG[:], dbg_sb[:])

                # interleaved precompute: one (g, q) chunk per step, split in
                # half around the transposes so they don't stall the PE queue
                pg, pq = PRE_AT + (k - 1) // 4, (k - 1) % 4

                # ---- elementwise
                asig = ep.tile([128, 384], dt.float32, name=f"as{k}", tag="asig")
                nc.scalar.activation(asig[:, 0:384], P_cur[:, 0:384], AF.Sigmoid)
                ta = ep.tile([128, 128], dt.float32, name=f"ta{k}", tag="ta")
                nc.scalar.activation(ta[:], P_cur[:, 384:512], AF.Tanh)
                last = k == nstep
                so = ep.tile([128, 128],
                             dt.float32 if last else dt.bfloat16,
                             name=f"so{k}", tag="so_f" if last else "so")
                nc.scalar.activation(so[:], P_cur[:, 512:640], AF.Sigmoid)

                m2 = ep.tile([128, 256], dt.float32, name=f"m2{k}", tag="m2")
                nc.vector.tensor_mul(m2[:], asig[:, 0:256], L[k][:, 0:256])
                m1 = ep.tile([128, 128], dt.float32, name=f"m1{k}", tag="m1")
                m1_eng = nc.gpsimd if CFG.get("m1_gps") else nc.vector
                m1_eng.tensor_mul(m1[:], ta[:], asig[:, 256:384])
                s1 = ep.tile([128, 128], dt.float32, name=f"s1{k}", tag="s1")
                nc.vector.tensor_add(s1[:], m2[:, 0:128], m2[:, 128:256])
                if not last:
                    c_dst = L[k + 1][:, 0:128]
                else:
                    c_fin = ep.tile([128, 128], dt.float32, name="c_fin", tag="cf")
                    c_dst = c_fin[:]
                nc.vector.tensor_add(c_dst, s1[:], m1[:])

                tc_t = ep.tile([128, 128],
                               dt.float32 if last else dt.bfloat16,
                               name=f"tc{k}", tag="tc_f" if last else "tc")
                nc.scalar.activation(tc_t[:], c_dst, AF.Tanh)

                if not last:
                    # PE: this step's A-chunk precompute, then the transposes;
                    # the B chunk + staging were deferred to the next step
                    if not pre_pro:
                        pre_a(pg, pq)
                    pstt = pst.tile([128, 256], dt.bfloat16, name=f"pst{k}", tag="pst")
                    nc.tensor.transpose(pstt[:, 0:128], so[:], id_t)
                    nc.tensor.transpose(pstt[:, 128:256], tc_t[:], id_t)
                    # keep-warm: free-running LDWEIGHTS churn occupies the PE
                    # through the tail window so the HAM clock-gate never
                    # re-throttles while waiting for the next hT
                    for i in range(int(CFG.get("kwl", 0))):
                        nc.tensor.ldweights(wl(i % 4, 0, 128))
                    soT = ep.tile([128, 128], dt.bfloat16, name=f"soT{k}", tag="soT")
                    nc.vector.tensor_copy(soT[:], pstt[:, 0:128])
                    ht = htp.tile([128, 128], dt.bfloat16, name=f"ht{k}", tag="ht")
                    nc.vector.tensor_mul(ht[:], pstt[:, 128:256], soT[:])
                    # timing ablation: break the recurrence to expose the
                    # pure PE-pipeline floor (numerically wrong)
                    hT_prev = lh0_t if CFG.get("no_rec") else ht
                    # evacuations ride at the DVE tail end
                    if not pre_pro:
                        if k >= 2:
                            evac_b(PRE_AT + (k - 2) // 4, (k - 2) % 4)
                        evac_a(pg, pq)
                else:
                    hf = ep.tile([128, 128], dt.float32, name="hf", tag="hf")
                    nc.vector.tensor_mul(hf[:], so[:], tc_t[:])
                    nc.sync.dma_start(OUT[:], hf[:])

        if repeat == 1:
            chain()
        else:
            with tc.For_i(0, repeat, 1):
                chain()

    nc.finalize()
    return nc


def _pack_inputs(buffers, Wl, Wr, bl):
    """Host-side re-layout into the kernel's tensor formats."""
    buffers = np.asarray(buffers, F32)
    Wl = np.asarray(Wl, F32)
    Wr = np.asarray(Wr, F32)
    bl = np.asarray(bl, F32)

    # weights: W [512, 2560] -> [kc, p, q*640 + gi*128 + v], gates permuted
    def arr_w(W):
        w = W.reshape(4, 128, 5, 4, 128)[:, :, GATE_PERM, :, :]
        return np.ascontiguousarray(w.transpose(0, 1, 3, 2, 4).reshape(4, 128, 2560))

    WA = np.concatenate([arr_w(Wl), arr_w(Wr)], axis=2).astype(BF16)
    blp = bl.reshape(5, 4, 128)[GATE_PERM].transpose(1, 0, 2).reshape(2560)
    BQ = np.tile(blp[None, :], (128, 1)).astype(BF16)

    ident = np.eye(128, dtype=F32)
    in_maps = []
    for c in range(NCORES):
        bc = slice(BPC * c, BPC * (c + 1))
        bh = buffers[bc, 1:, :SIZE]          # [32, 63, 512]
        bcc = buffers[bc, 1:, SIZE:]         # [32, 63, 512]
        # RB[g, kc, p, 32s+b] = bh[b, 4g+s, kc*128+p]
        bh2 = np.zeros((BPC, 4 * NGRP, 4, 128), F32)
        bh2[:, :NSTEP] = bh.reshape(BPC, NSTEP, 4, 128)
        RBc = bh2.reshape(BPC, NGRP, 4, 4, 128).transpose(1, 3, 4, 2, 0) \
                 .reshape(NGRP, 4, 128, 128)
        # RC[k, 32q+b, v] = bcc[b, k, q*128+v]
        RCc = bcc.reshape(BPC, NSTEP, 4, 128).transpose(1, 2, 0, 3).reshape(NSTEP, 128, 128)
        h0 = buffers[bc, 0, :SIZE]           # [32, 512]
        lh0T = h0.reshape(BPC, 4, 128).transpose(2, 1, 0).reshape(128, 128)
        c0 = buffers[bc, 0, SIZE:]
        C0c = c0.reshape(BPC, 4, 128).transpose(1, 0, 2).reshape(128, 128)
        in_maps.append({
            "RB": np.ascontiguousarray(RBc).astype(BF16),
            "RC": np.ascontiguousarray(RCc),
            "CB": np.concatenate([lh0T, ident], axis=1).astype(BF16),
            "C0": np.ascontiguousarray(C0c),
            "WA": WA,
            "BQ": BQ,
        })
    return in_maps


def _run(in_maps, trace=False, **kw):
    from concourse.bass_utils import run_bass_kernel_spmd

    key = ("prog", tuple(sorted(CFG.items())))
    if key not in _CACHE:
        _CACHE[key] = _build_program()
    nc = _CACHE[key]
    return run_bass_kernel_spmd(nc, in_maps, list(range(NCORES)), trace=trace, **kw)


def kernel(buffers, transitions, Wl, Wr, bl):
    transitions = np.asarray(transitions)
    if transitions.shape != (B, T) or np.asarray(buffers).shape != (B, NTOK, 2 * SIZE) \
            or not np.array_equal(transitions, _expected_transitions()):
        # input doesn't match the compiled left-chain schedule: exact fallback
        return _numpy_fallback(buffers, transitions, Wl, Wr, bl)

    CFG.clear()
    CFG["kwl"] = 2
    if np.any(np.asarray(bl)):
        CFG["bias"] = True
    in_maps = _pack_inputs(buffers, Wl, Wr, bl)
    res = _run(in_maps)
    out = np.empty((B, SIZE), F32)
    for c in range(NCORES):
        oc = res.results[c]["out"]  # [128, 128] = [(q,b), v]
        out[BPC * c:BPC * (c + 1)] = \
            oc.reshape(4, BPC, 128).transpose(1, 0, 2).reshape(BPC, SIZE)
    return out


if __name__ == "__main__":
    import reference as ref

    inputs = ref.setup_inputs()
    np_in = {k: np.asarray(v) for k, v in inputs.items()}
    got = kernel(**np_in)
    exp = np.asarray(ref.reference(**inputs))
    num = np.linalg.norm(got - exp)
    den = np.linalg.norm(exp)
    print("rms rel err:", num / den)
    print("absmax diff:", np.abs(got - exp).max())
